# revision 1
# baseline (speedup 1.0000x reference)
"""Trainium2 Bass kernel for the circular drift-diffusion loss (batched expm).

Reference computes  loss = -mean_b log(relu(e_{idx_b}^T expm(t_b*A) p0_b) + eps)
with A a fixed 360x360 circular advection-diffusion operator, t_b in [0,1000),
p0_b a von Mises density, over a batch of 256.

Algorithm (per core; batch sharded 32/core over 8 cores):
  * Quantize t_b = m_b*T0 + r_b with T0 = 1000/2^K, m_b < 2^K.
  * Build propagator chain M_j = expm(2^j*T0*A) once by repeated squaring
    (prelude: ascending Taylor at T0, then K-2 squarings).  Level j also
    applies bit j of the quantized delay to the per-sample state Q with
    small f32 matmuls that fill the PE bubble before the transposes, and
    one masked wide predicated copy blends the result.
  * All wide matmuls run as float32r: 1 PE cycle/row (vs fp32's 4) at
    ~1.6e-4 relative precision - ample against the 2e-2 harness gate.
    Transposes batch 3 PE-transpose blocks per PSUM bank; their strided
    copies land exactly the columns the next level's matmul group needs.
  * The last chain level only needs M^T (as matmul lhsT for the final bit
    applies), so it squares in transpose space (MT@MT via lhsT=M).  Bit
    K-2 is a single apply of M_{K-2}; the top bit K-1 is folded into the
    selection as two dot-product rows (w^T M^2 Q vs w^T Q) picked by a
    tiny [1,BL] predicated copy, so no wide blend sits on the tail.
  * The residual Taylor(r_b*A) commutes with the chain, so it is applied
    on the SELECTION side: w_b = Taylor(r_b*A^T) onehot_b (Horner with
    host-folded r/k tables, lhsT=X), interleaved into the chain levels'
    PE slack; psel = w^T (chain Q).  Loss terms via exponent/mantissa
    split + Ln activation (table preloaded by an early dummy Ln).
  * Engine rules discovered on HW: only DVE/ACT read PSUM; every producer
    feeding an f32r matmul must be f32r-typed; copy_predicated cannot
    write f32r (hence the separate f32 Q path).
Everything O(n^2)+ per-sample work runs on device; host does operator
assembly (replicating the reference's f32 evo_mat construction), the
spectral-free plan selection, and index/bit/p0/one-hot layout glue.
"""

import math

import numpy as np

# ---------------- static problem constants (hardcoded per contract) ----------
N = 360            # color mesh size
P = 120            # partition chunk (N = 3*P)
NCH = 3            # chunks
B = 256            # total batch
NCORES = 8
BL = B // NCORES   # per-core batch
T_MAX = 1000.0
KAPPA = 400.0      # 1/SIGMA_INIT^2
EPS = 1e-5
TWO_PI = 6.283185307179586
# ln(1/(2*pi*i0e(400)))  [i0e(400) = 0.019953356281939987]
LNC = 2.076480848703078
# minimax fit of cos(sqrt(u)) on u in [0, pi^2], power basis c0..c6
# (|delta| folded to [0,pi]; max abs err 1.1e-8)
COS_COEF = [0.9999999891, -0.499999891, 0.04166648922, -0.00138878036,
            2.476988354e-05, -2.707903089e-07, 1.724509195e-09]

_COMPILED = {}


def _taylor_deg(x, tol, lo):
    """Smallest d with x^(d+1)/(d+1)! < tol."""
    d = lo
    term = x ** (d + 1) / math.factorial(d + 1)
    while term > tol and d < 40:
        d += 1
        term *= x / (d + 1)
    return d


def _plan(anorm):
    """Choose (k_bits, deg_p, deg_r) from ||A||_inf.  The time grid is
    T0 = T_MAX/2^k_bits; every squaring level applies one bit of the
    quantized delay.  Cost constants (ns) re-measured for the f32r kernel:
    chain level ~2.9us, prelude step ~1.6us, residual taylor interleaves
    (cheap, mostly hidden)."""
    xa = T_MAX * float(anorm)
    if xa <= 0.0:
        return 2, 4, 3

    def degrees(k):
        x0 = xa / (1 << k)
        # prelude truncation amplifies roughly 2^(k-3) worst-case through
        # the squarings; budget ~1e-3 of the 2e-2 gate for it (the f32r
        # matmul rounding already contributes ~3e-3).  The residual Taylor
        # is applied once (no amplification) and tolerates more.
        tol_p = min(max(1.6e-1 / 2 ** (k - 3), 5e-8), 1.0e-2)
        return _taylor_deg(x0, tol_p, 2), _taylor_deg(x0, 1.5e-1, 1)

    best = None
    for k in range(2, 17):
        x0 = xa / (1 << k)
        if x0 > 2.2 and k < 16:   # keep the ascending Taylor numerically tame
            continue
        dp, dr = degrees(k)
        # measured: chain level ~3.2us, prelude stage ~2.05us; residual
        # taylor steps mostly hide in chain/tail slack
        cost = (k - 2) * 3.2 + (dp - 1) * 2.05 + dr * 0.12
        if dr * 0.55 > (dp - 1) * 2.05:
            cost += dr * 0.55 - (dp - 1) * 2.05
        if best is None or cost < best[0]:
            best = (cost, k, dp, dr)
    _, k, deg_p, deg_r = best
    return k, deg_p, deg_r


def _build_bass(k_bits, deg_p, deg_r):
    """Construct the Bass program (SPMD; identical on all 8 cores).

    Engine ground rules (BIR-verified): only DVE and ACT can read PSUM;
    anything feeding a float32r matmul must itself be declared float32r
    (DMA, copies, adds all work); copy_predicated cannot write f32r, so
    the per-sample state Q stays f32 in separate tiles and each chain
    level applies its bit with small f32 matmuls (lhsT bitcast from the
    f32r M^T) that also fill the PE bubble before the transposes.
    """
    import concourse.tile as tile
    from concourse import bacc, mybir

    F32 = mybir.dt.float32
    F32R = mybir.dt.float32r
    AF = mybir.ActivationFunctionType
    OP = mybir.AluOpType

    nc = bacc.Bacc("TRN2", target_bir_lowering=False, debug=False)

    d_x = nc.dram_tensor("x", [N, N], F32R, kind="ExternalInput").ap()
    d_xt = nc.dram_tensor("xt", [N, N], F32R, kind="ExternalInput").ap()
    # packed small inputs: [q0(NCH*BL) | rdk(deg_r*BL) | oh(NCH*BL)]
    PKW = NCH * BL + deg_r * BL + NCH * BL
    d_pk = nc.dram_tensor("pk", [P, PKW], F32, kind="ExternalInput").ap()
    d_msk = nc.dram_tensor("msk", [P, k_bits * NCH * BL], mybir.dt.uint8,
                           kind="ExternalInput").ap()
    d_eye = nc.dram_tensor("eye", [P, P], F32R, kind="ExternalInput").ap()
    d_out = nc.dram_tensor("terms", [1, BL], F32, kind="ExternalOutput").ap()

    with tile.TileContext(nc) as tc:
        with (
            tc.tile_pool(name="const", bufs=1) as cpool,
            tc.tile_pool(name="mats", bufs=4) as mpool,
            tc.tile_pool(name="qp", bufs=2) as qpool,
            tc.tile_pool(name="vp", bufs=3) as vpool,
            tc.tile_pool(name="tp", bufs=4) as tpool,
            tc.tile_pool(name="psb", bufs=3, space="PSUM") as psb,
            tc.tile_pool(name="pst", bufs=3, space="PSUM") as pstp,
            tc.tile_pool(name="pss", bufs=2, space="PSUM") as pss,
        ):
            # ---- constants ------------------------------------------------
            ONES = cpool.tile([P, 1], F32, tag="ones")
            nc.vector.memset(ONES[:], 1.0)
            BLN0 = cpool.tile([1, 1], F32, tag="bln0")
            nc.vector.memset(BLN0[:], 0.0)
            WU = cpool.tile([P, 240], F32, tag="wu")
            nc.vector.memset(WU[:], 0.5)
            E120 = cpool.tile([P, P], F32R, tag="e120")
            # dummy Ln as the FIRST ACT activation: loads the ln table set
            # once, up front; later ACT Copy ops never force a switch, so the
            # final Ln needs no load on the critical tail.
            DUMLN = tpool.tile([1, 1], F32, tag="dl")
            nc.scalar.activation(DUMLN[:], ONES[:1, :], AF.Ln, bias=BLN0[:],
                                 scale=1.0)
            XN = cpool.tile([P, NCH * N], F32R, tag="x")
            XT = cpool.tile([P, NCH * N], F32R, tag="xt")
            # queue plan: SP[XT0, XN0, PK] | ACT[XT1, XN1, MSK] | Pool-SWDGE
            # [XT2, XN2] -- chunk-c DMAs land in first-matmul use order
            nc.sync.dma_start(XT[:, 0:N], d_xt[0:P, :])
            nc.scalar.dma_start(XT[:, N:2 * N], d_xt[P:2 * P, :])
            nc.gpsimd.dma_start(XT[:, 2 * N:3 * N], d_xt[2 * P:3 * P, :])
            nc.sync.dma_start(XN[:, 0:N], d_x[0:P, :])
            nc.scalar.dma_start(XN[:, N:2 * N], d_x[P:2 * P, :])
            nc.gpsimd.dma_start(XN[:, 2 * N:3 * N], d_x[2 * P:3 * P, :])
            nc.gpsimd.dma_start(E120[:], d_eye)
            PK = cpool.tile([P, PKW], F32, tag="pk")
            nc.sync.dma_start(PK[:], d_pk[:])
            MSK = cpool.tile([P, k_bits * NCH * BL], mybir.dt.uint8,
                             tag="msk")
            nc.scalar.dma_start(MSK[:], d_msk[:])
            Q0 = PK[:, 0:NCH * BL]
            RDK = PK[:, NCH * BL:(NCH + deg_r) * BL]
            OH = PK[:, (NCH + deg_r) * BL:PKW]

            def mm_group(out_ap, lhsT_tile, rhs_tile, i, rhs_w,
                         rhs_stride=None, f32lhs=False):
                """out_ap = sum_c lhsT_c^T @ rhs_c over the 3 row chunks."""
                rs = rhs_w if rhs_stride is None else rhs_stride
                for c in range(NCH):
                    lh = lhsT_tile[:, c * N + i * P: c * N + i * P + P]
                    if f32lhs:
                        lh = lh.bitcast(F32)
                    nc.tensor.matmul(out_ap, lhsT=lh,
                                     rhs=rhs_tile[:, c * rs: c * rs + rhs_w],
                                     start=(c == 0), stop=(c == NCH - 1))

            def cp_dve(d, s):
                nc.vector.tensor_copy(d, s)

            def cp_act(d, s):
                nc.scalar.copy(d, s)

            # ---- PE warmup -------------------------------------------------
            # The PE p-state needs ~3us of continuous execution to reach full
            # clock; burn the ramp on throwaway matmuls with no DMA deps.
            I32 = mybir.dt.int32
            for _ in range(4):
                wps = psb.tile([1, 240], F32, tag="sq")
                nc.tensor.matmul(wps[:], lhsT=ONES[:], rhs=WU[:],
                                 start=True, stop=True)

            # ---- residual Taylor, commuted onto the SELECTION side:
            # psel = OH^T Taylor(rA) (Mchain p0) = [Taylor(rA^T) OH]^T
            # (Mchain p0).  The w-chain w = OH + rdk_k*(X^T w) uses the
            # constant one-hot vectors, so its serial steps interleave into
            # the chain levels' PE slack instead of gating the prelude/Q.
            taylor_state = {"V": OH, "k": deg_r, "res": OH}

            def taylor_step():
                k = taylor_state["k"]
                if k < 1:
                    return
                Vc = taylor_state["V"]
                Vn = vpool.tile([P, NCH * BL], F32, tag="V")
                for i in range(NCH):
                    ps = pss.tile([P, BL], F32, tag="ap")
                    mm_group(ps[:], XN, Vc, i, BL, f32lhs=True)
                    vs = Vn[:, i * BL:(i + 1) * BL]
                    nc.vector.tensor_tensor(
                        vs, ps[:], RDK[:, (k - 1) * BL: k * BL], op=OP.mult)
                    nc.gpsimd.tensor_tensor(
                        vs, vs, OH[:, i * BL:(i + 1) * BL], op=OP.add)
                taylor_state["V"] = Vn
                taylor_state["k"] = k - 1
                taylor_state["res"] = Vn

            # ---- prelude: ascending Taylor S = I + sum X^k/k! -------------
            S = mpool.tile([P, NCH * N], F32R, tag="M")
            for c in range(NCH):
                eng = nc.gpsimd if c == 1 else nc.vector
                eng.tensor_copy(S[:, c * N:(c + 1) * N],
                                XN[:, c * N:(c + 1) * N])
            for c in range(NCH):
                dg = slice(c * N + c * P, c * N + (c + 1) * P)
                eng = nc.gpsimd if c == 1 else nc.vector
                eng.tensor_tensor(S[:, dg], S[:, dg], E120[:], op=OP.add)
            T = XN
            for k in range(2, deg_p + 1):
                Tn = mpool.tile([P, NCH * N], F32R, tag="T")
                for i in range(NCH):
                    ps = psb.tile([P, N], F32, tag="sq")
                    mm_group(ps[:], XT, T, i, N)
                    # evacuation first (gates the next stage): chunk0 on
                    # DVE, chunks 1/2 on ACT so DVE isn't a serial backlog
                    if i == 0:
                        nc.vector.tensor_scalar(Tn[:, 0:N], ps[:], 1.0 / k,
                                                None, op0=OP.mult)
                    else:
                        nc.scalar.mul(Tn[:, i * N:(i + 1) * N], ps[:], 1.0 / k)
                # S-accumulates off the critical path: add0 right behind
                # evac0 on DVE (gates the first transpose group at prelude
                # end), add1 on Pool, add2 on DVE last
                nc.vector.tensor_tensor(S[:, 0:N], S[:, 0:N], Tn[:, 0:N],
                                        op=OP.add)
                nc.gpsimd.tensor_tensor(S[:, N:2 * N], S[:, N:2 * N],
                                        Tn[:, N:2 * N], op=OP.add)
                nc.vector.tensor_tensor(S[:, 2 * N:], S[:, 2 * N:],
                                        Tn[:, 2 * N:], op=OP.add)
                T = Tn

            ST = mpool.tile([P, NCH * N], F32R, tag="MT")

            def tr_group(pt, Mt, ib):
                for cp in range(NCH):
                    nc.tensor.transpose(
                        pt[:, cp * P:(cp + 1) * P],
                        Mt[:, ib * N + cp * P: ib * N + cp * P + P],
                        E120[:],
                    )

            def mt_copy(eng, MTt, pt, ib):
                """Strided copy: MTt cols [ib*P,+P) of every dest chunk --
                exactly what the next level's matmul group i=ib consumes."""
                MT3 = MTt[:].rearrange("p (c n) -> p c n", c=NCH)
                eng(MT3[:, :, ib * P:(ib + 1) * P],
                    pt[:].rearrange("p (c n) -> p c n", c=NCH))

            def transpose_mq(MTt, Mt):
                for ib in range(NCH):
                    pt = pstp.tile([P, N], F32R, tag="tr")
                    tr_group(pt, Mt, ib)
                    mt_copy(cp_dve if ib != 1 else cp_act, MTt, pt, ib)

            transpose_mq(ST, S)
            M, MT = S, ST
            Qcur = Q0

            QW = NCH * BL

            def apply_mms(MTc, q_rhs):
                """M @ q for all 3 output chunks into ONE [P, 3*BL] psum."""
                ps = pss.tile([P, QW], F32, tag="ap")
                for i in range(NCH):
                    mm_group(ps[:, i * BL:(i + 1) * BL], MTc, q_rhs, i,
                             BL, f32lhs=True)
                return ps

            def blend_base(blend_src):
                Qn = qpool.tile([P, QW], F32, tag="q")
                nc.gpsimd.tensor_copy(Qn[:], blend_src[:, 0:QW])
                return Qn

            def blend_pred(Qn, ps, bit):
                nc.vector.copy_predicated(
                    Qn[:], MSK[:, bit * QW:(bit + 1) * QW], ps[:])
                return Qn

            def apply_bit(MTc, q_rhs, bit, blend_src):
                # base copy first (no psum dep) so the predicated write is
                # the only op waiting on the matmuls
                if bit is not None:
                    Qn = blend_base(blend_src)
                ps = apply_mms(MTc, q_rhs)
                if bit is None:
                    return ps
                return blend_pred(Qn, ps, bit)

            def square(Mc, MTc, bit, Qc):
                """Sn = Mc@Mc (f32r, wide); Qn = bit ? Mc@Qc : Qc (f32,
                narrow -- the apply matmuls run in the PE bubble between the
                squaring matmuls and the transposes).  Emission order on DVE
                puts the mcp strided copies (which gate the next level's
                matmul groups) before the Q blends (which only gate the next
                level's small applies)."""
                Sn = mpool.tile([P, NCH * N], F32R, tag="M")
                STn = mpool.tile([P, NCH * N], F32R, tag="MT")
                for i in range(NCH):
                    ps = psb.tile([P, N], F32, tag="sq")
                    mm_group(ps[:], MTc, Mc, i, N)
                    if i == 0:
                        cp_dve(Sn[:, 0:N], ps[:])
                    elif i == 1:
                        cp_act(Sn[:, N:2 * N], ps[:])
                    else:
                        cp_dve(Sn[:, 2 * N:2 * N + N // 2], ps[:, :N // 2])
                        cp_act(Sn[:, 2 * N + N // 2:3 * N], ps[:, N // 2:])
                # bit apply matmuls in the PE bubble before the transposes
                Qn = blend_base(Qc)
                aps = apply_mms(MTc, Qc)
                pt0 = pstp.tile([P, N], F32R, tag="tr")
                tr_group(pt0, Sn, 0)
                mt_copy(cp_dve, STn, pt0, 0)
                pt1 = pstp.tile([P, N], F32R, tag="tr")
                tr_group(pt1, Sn, 1)
                mt_copy(cp_act, STn, pt1, 1)
                pt2 = pstp.tile([P, N], F32R, tag="tr")
                tr_group(pt2, Sn, 2)
                mt_copy(cp_dve, STn, pt2, 2)
                # predicated overwrite last (base already copied)
                blend_pred(Qn, aps, bit)
                return Sn, STn, Qn

            # ---- chain: level j squares M_j and applies bit j to Q --------
            # w-steps ride the chain only down to k=3: the levels are
            # PE-bound, while the last-level/apply region below has PE
            # bubbles (waiting on MT2 copies and blends) that hide the
            # final two steps for free.
            for j in range(k_bits - 3):
                M, MT, Qcur = square(M, MT, j, Qcur)
                if taylor_state["k"] > 1:
                    taylor_step()

            # ---- last chain level: only M^T is needed (as lhsT for the
            # final applies), so square in transpose space (MT@MT via
            # lhsT=M) and apply bit k-3 on the side.
            if k_bits >= 3:
                Qa = apply_bit(MT, Qcur, k_bits - 3, Qcur)
                MT2 = mpool.tile([P, NCH * N], F32R, tag="MT")
                for i in range(NCH):
                    ps = psb.tile([P, N], F32, tag="sq")
                    mm_group(ps[:], M, MT, i, N)
                    h = N // 2
                    cp_dve(MT2[:, i * N:i * N + h], ps[:, :h])
                    cp_act(MT2[:, i * N + h:(i + 1) * N], ps[:, h:])
                MT = MT2
                taylor_step()   # hides while MT2 copies drain
                # bit k-2: single apply of M_{k-2}
                Qf = apply_bit(MT, Qa, k_bits - 2, Qa)
            else:
                Qf = apply_bit(MT, Qcur, k_bits - 2, Qcur)
            taylor_step()       # hides while the Qf blend drains
            # drain any remaining w-steps (k_bits-3 levels might be
            # fewer than deg_r)
            while taylor_state["k"] >= 1:
                taylor_step()
            WSEL = taylor_state["res"]

            # ---- top bit k-1 folded into the selection --------------------
            # psel = b_{k-1} ? w^T(M@(M@Qf)) : w^T Qf.  Both dot products
            # are cheap [1,BL] rows: the b=0 branch runs entirely off the
            # critical path right after Qf, the b=1 branch multiplies the
            # apply PSUM directly (no V materialization, no wide blend),
            # and a tiny predicated copy picks per sample at the end.
            tmpb = tpool.tile([P, NCH * BL], F32, tag="t3")
            nc.gpsimd.tensor_tensor(tmpb[:], Qf[:, 0:NCH * BL],
                                    WSEL[:, 0:NCH * BL], op=OP.mult)
            # bit k-1: double apply of the same M
            y1ps = apply_mms(MT, Qf)
            Y1 = qpool.tile([P, NCH * BL], F32, tag="y")
            cp_act(Y1[:], y1ps[:])
            vps = apply_mms(MT, Y1)
            # both dot products after the applies so the tiny row matmuls
            # never delay the critical apply chain on the in-order PE
            selb = pss.tile([1, BL], F32, tag="ap")
            for c in range(NCH):
                nc.tensor.matmul(selb[:], lhsT=ONES[:],
                                 rhs=tmpb[:, c * BL:(c + 1) * BL],
                                 start=(c == 0), stop=(c == NCH - 1))
            SELB = tpool.tile([1, BL], F32, tag="sb")
            cp_dve(SELB[:], selb[:])
            tmpa = tpool.tile([P, NCH * BL], F32, tag="t4")
            nc.vector.tensor_tensor(tmpa[:], vps[:], WSEL[:, 0:NCH * BL],
                                    op=OP.mult)
            sela = pss.tile([1, BL], F32, tag="ap")
            for c in range(NCH):
                nc.tensor.matmul(sela[:], lhsT=ONES[:],
                                 rhs=tmpa[:, c * BL:(c + 1) * BL],
                                 start=(c == 0), stop=(c == NCH - 1))
            nc.vector.copy_predicated(
                SELB[:],
                MSK[0:1, (k_bits - 1) * QW:(k_bits - 1) * QW + BL],
                sela[:])
            # ln(relu(psel)+eps) via exponent/mantissa split: the HW Ln
            # table degrades for huge args (psel can reach ~1e20 in the
            # weak-diffusion regime), so compute ln(m) + e*ln2 with m in
            # [1,2), which keeps the table in its accurate range.  Integer
            # ops run back-to-back on DVE; ACT's Ln overlaps.
            rl = tpool.tile([1, BL], F32, tag="r0")
            nc.vector.tensor_scalar(rl[:], SELB[:], 0.0, EPS,
                                    op0=OP.max, op1=OP.add)
            xi = rl[:].bitcast(I32)
            mi = tpool.tile([1, BL], I32, tag="r4")
            nc.vector.tensor_scalar(mi[:], xi, 0x007FFFFF, 0x3F800000,
                                    op0=OP.bitwise_and, op1=OP.bitwise_or)
            lnm = tpool.tile([1, BL], F32, tag="r5")
            nc.scalar.activation(lnm[:], mi[:].bitcast(F32), AF.Ln,
                                 bias=BLN0[:], scale=1.0)
            et = tpool.tile([1, BL], I32, tag="r2")
            nc.vector.tensor_scalar(et[:], xi, 23, None,
                                    op0=OP.arith_shift_right)
            ef = tpool.tile([1, BL], F32, tag="r3")
            nc.vector.tensor_copy(ef[:], et[:])
            terms = tpool.tile([1, BL], F32, tag="r1")
            # ef holds the biased exponent; fold the -127*ln2 into the mult
            nc.vector.tensor_scalar(terms[:], ef[:], 0.6931471805599453,
                                    -88.02969193111305,
                                    op0=OP.mult, op1=OP.add)
            nc.vector.tensor_tensor(terms[:], terms[:], lnm[:], op=OP.add)
            nc.sync.dma_start(d_out[:], terms[:])

    nc.compile()
    return nc


def _host_prep(c_mesh, gtheta, sigma_diff, init_color, delay_t, report_color):
    """Host-side glue: operator assembly (replicating reference f32 ops),
    plan selection, and per-core index/bit/layout arrays."""
    f32 = np.float32
    c = np.asarray(c_mesh, dtype=f32)
    g = np.asarray(gtheta, dtype=f32)
    s = np.asarray(sigma_diff, dtype=f32)[0]
    init = np.asarray(init_color, dtype=f32)
    t = np.asarray(delay_t, dtype=f32)
    rep = np.asarray(report_color, dtype=f32)

    d = (c[1] - c[0]).astype(f32)
    eye = np.eye(N, dtype=f32)
    up = np.roll(eye, -1, axis=1)
    dn = np.roll(eye, 1, axis=1)
    D1 = ((up - dn) / (f32(2.0) * d)).astype(f32)
    D2 = ((up - f32(2.0) * eye + dn) / (d * d)).astype(f32)
    A = ((s ** f32(2.0)) / f32(2.0) * D2 - D1 * g[None, :]).astype(f32)

    anorm = np.abs(A.astype(np.float64)).sum(axis=1).max()
    k_bits, deg_p, deg_r = plan = _plan(anorm)
    T0 = T_MAX / (1 << k_bits)
    X = (A * f32(T0)).astype(f32)

    m = np.floor(t.astype(np.float64) / T0).astype(np.int64)
    m = np.clip(m, 0, (1 << k_bits) - 1)
    r = (t.astype(np.float64) - m * T0) / T0  # in X = T0*A units
    bits = ((m[:, None] >> np.arange(k_bits)[None, :]) & 1)     # [B, K]
    idx = np.argmin(np.abs(c[None, :] - rep[:, None]), axis=1)

    # von Mises p0 (matches the reference's f32 evaluation: cos/exp in f64
    # then cast, normalization folded in)
    i0e400 = 0.019953356281939987
    z = np.cos(c[None, :].astype(np.float64) - init[:, None].astype(np.float64)) - 1.0
    p0 = (np.exp(KAPPA * z) / (2.0 * np.pi * i0e400)).astype(f32)  # [B, N]

    shared = {
        "x": X,
        "xt": np.ascontiguousarray(X.T),
        "eye": np.eye(P, dtype=f32),
    }
    in_maps = []
    for core in range(NCORES):
        sl = slice(core * BL, (core + 1) * BL)
        # bit j occupies [j*3*BL:(j+1)*3*BL], replicated across the 3
        # mesh chunks so blends are a single wide predicated copy
        mskb = np.tile(bits[sl].T[:, None, :], (1, NCH, 1))  # [K, NCH, BL]
        msk = np.broadcast_to(
            mskb.reshape(1, k_bits * NCH * BL), (P, k_bits * NCH * BL)
        ).astype(np.uint8)
        # q0[p, c*BL+b] = p0[b, c*P+p]
        q0 = np.ascontiguousarray(
            p0[sl].reshape(BL, NCH, P).transpose(2, 1, 0).reshape(P, NCH * BL)
        ).astype(f32)
        rdk = np.empty((deg_r, BL), f32)
        for k in range(1, deg_r + 1):
            rdk[k - 1] = (r[sl] / k).astype(f32)
        rdk = np.broadcast_to(
            rdk.reshape(1, deg_r * BL), (P, deg_r * BL)).astype(f32)
        oh = np.zeros((NCH, P, BL), f32)
        for b, ix in enumerate(idx[sl]):
            oh[ix // P, ix % P, b] = 1.0
        oh = np.ascontiguousarray(oh.transpose(1, 0, 2).reshape(P, NCH * BL))
        pk = np.ascontiguousarray(
            np.concatenate([q0, rdk, oh], axis=1)).astype(f32)
        in_maps.append(dict(shared, pk=pk, msk=msk))
    return plan, in_maps


def _get_nc(plan):
    if plan not in _COMPILED:
        _COMPILED[plan] = _build_bass(*plan)
    return _COMPILED[plan]


def kernel(**inputs):
    from concourse.bass_utils import run_bass_kernel_spmd

    plan, in_maps = _host_prep(
        inputs["c_mesh"], inputs["gtheta"], inputs["sigma_diff"],
        inputs["init_color"], inputs["delay_t"], inputs["report_color"],
    )
    nc = _get_nc(plan)
    res = run_bass_kernel_spmd(nc, in_maps, list(range(NCORES)))
    terms = np.concatenate(
        [np.asarray(res.results[k]["terms"]).reshape(-1) for k in range(NCORES)]
    )
    loss = -np.mean(terms.astype(np.float64))
    return np.asarray(loss, dtype=np.float32)

